# revision 20
# baseline (speedup 1.0000x reference)
"""MultiHeadMixer Trainium2 kernel (v4 = proven v1 skeleton + partial fp8).

Math (matches the reference):
  x: (B=8, E=1024, S=2048), weight: (H=16, S), bias: (H, S)
  out[m, r, t] = sum_{s<=t} xg[m, r, s] * weight[m, t-s] + bias[m, t]

Per head C = A @ M with M upper-triangular Toeplitz; every 128x128 block of
M depends only on d = t_tile - s_tile and is a column slice of the (128, S)
shifted-weight array T, T[i, c] = w[c - i].  Device computation per head:
  psum[tb] += T(d).T @ xT[k],  d = tb - k >= 0
with xT the transposed x staged per 128-row s-tile (moving free dim N=512).
136 upper-triangular blocks per head; lower-triangle blocks skipped.

v4 vs v1 (76.9us):
  * Blocks k in {2,3} for tb >= 4 run as one fp8e4 DoubleRow matmul per
    output tile (2 blocks per 512-cycle matmul = 2x MAC rate).  24/136 of
    the MACs in fp8 -> rel err ~1.6e-2 (gate 2e-2), saves 12 matmuls/head
    (~5.2us of PE time).  The fp8 matmul doubles as each tile's start in
    g1..g3 so it carries only the psum-bank wait.
  * Sync DMA order tuned (x h0 k1-3 right after comb_a h0, comb_a h1
    earlier) to close v1's early supply gaps.  All DMAs stay on the Sync
    queue: spreading them over Act/GpSimd queues starves the DMA-semaphore
    pool and serializes on recycle waits (v2/v3 regressions).
  * Tail: the last output tile drains as two 256-col halves, one on the
    Act engine (which issues its own output DMA back-to-back), one on DVE.

Sharding: head-parallel, 2 heads per core across 8 cores, no collectives.
"""

import numpy as np
import ml_dtypes

import concourse.bass as bass
import concourse.mybir as mybir
import concourse.tile as tile
from concourse import bacc
from concourse.bass_utils import run_bass_kernel_spmd

B, E, S, H = 8, 1024, 2048, 16
D = E // H            # 64   per-head hidden dim
NCORES = 8
HPC = H // NCORES     # 2    heads per core
R = B * D             # 512  rows per head (moving free dim)
KT = S // 128         # 16   128-wide tiles along the sequence axis
GRP = 4               # output tiles per PSUM group (4 banks per head)

DT = mybir.dt.float16
F8 = mybir.dt.float8e4
F32 = mybir.dt.float32
NPDT = np.float16
NP8 = ml_dtypes.float8_e4m3

import os as _os
FP8_MIX = _os.environ.get("FP8_MIX", "1") == "1"
FP8_KS = (2, 3)           # fp8 covers k in FP8_KS for tb >= 4

CA = GRP * 128 + R        # [Td0..3 | x0]
CB = CA + 32              # +32 fp16 cols holding the fp32 bias bits
CW = CB + (KT - GRP) * 128   # + Td4..15

_CACHED_NC = None


def _ensure_axon_hooks_stub():
    try:
        import antenv.axon_hooks  # noqa: F401
    except ImportError:
        import sys
        import types
        import antenv
        mod = types.ModuleType("antenv.axon_hooks")
        mod._hook = None
        mod.set_axon_ntff_profile_hook = lambda h: setattr(mod, "_hook", h)
        mod.get_axon_ntff_profile_hook = lambda: mod._hook
        sys.modules["antenv.axon_hooks"] = mod
        antenv.axon_hooks = mod


_ensure_axon_hooks_stub()


def _build_bass():
    nc = bacc.Bacc()
    xt = nc.dram_tensor("xt", [HPC, S, R], DT, kind="ExternalInput")
    comb = nc.dram_tensor("comb", [HPC, 128, CW], DT, kind="ExternalInput")
    if FP8_MIX:
        f8c = nc.dram_tensor("f8c", [HPC, 128, S + 2 * R], F8,
                             kind="ExternalInput")
    out = nc.dram_tensor("out", [HPC, KT // GRP, 128, GRP * R], DT,
                         kind="ExternalOutput")

    with tile.TileContext(nc) as tc:
        with (
            tc.tile_pool(name="xp", bufs=HPC * 4) as xp,
            tc.tile_pool(name="tp", bufs=HPC) as tp,
            tc.tile_pool(name="fp", bufs=HPC) as fpp,
            tc.tile_pool(name="op", bufs=HPC * (KT // GRP)) as op,
            tc.tile_pool(name="sp", bufs=1) as sp,
            tc.tile_pool(name="psA", bufs=GRP, space="PSUM") as psA,
            tc.tile_pool(name="psB", bufs=GRP, space="PSUM") as psB,
        ):
            # PE warm-up: HAM clock gate needs ~3.4us of sustained PE
            # activity; the PE idles ~10us waiting for the first DMA anyway.
            warm = sp.tile([128, R + 128], DT, tag="warm", name="warm")
            nc.gpsimd.memset(warm[:], 0)
            wps = psA.tile([128, R], F32, tag="acc", name="warm_ps")
            NWARM = 7
            for i in range(NWARM):
                nc.tensor.matmul(wps[:], warm[:, R:R + 128], warm[:, 0:R],
                                 start=(i == 0), stop=(i == NWARM - 1))

            combs, f8cs, xs = [], [], []
            for h in range(HPC):
                combs.append(tp.tile([128, CW], DT, tag="T",
                                     name=f"comb{h}"))
                if FP8_MIX:
                    f8cs.append(fpp.tile([128, S + 2 * R], F8, tag="T8",
                                         name=f"f8c{h}"))
                xs.append({0: combs[h][:, GRP * 128:CA]})

            def load_x(h, k0, k1):
                wd = (k1 - k0) * R
                xb = xp.tile([128, wd], DT, tag="x4", name=f"xb{h}_{k0}")
                src = xt[h, k0 * 128:k1 * 128, :].rearrange(
                    "(k p) r -> p k r", p=128)
                dst = xb[:].rearrange("p (k r) -> p k r", k=k1 - k0)
                nc.sync.dma_start(dst, src)
                for k in range(k0, k1):
                    xs[h][k] = xb[:, (k - k0) * R:(k - k0 + 1) * R]

            # All input DMAs on the Sync queue, ordered by consumption time.
            # bias rides inside comb_a as fp32 bits in fp16 columns: two
            # extra tiny DMAs here cost ~0.9us of early issue time each.
            nc.sync.dma_start(combs[0][:, :CB], comb[0, :, :CB])
            load_x(0, 1, 4)
            nc.sync.dma_start(combs[1][:, :CB], comb[1, :, :CB])
            if FP8_MIX:
                nc.sync.dma_start(f8cs[0][:], f8c[0])
            nc.sync.dma_start(combs[0][:, CB:], comb[0, :, CB:])
            load_x(1, 1, 4)
            load_x(0, 4, 8)
            if FP8_MIX:
                nc.sync.dma_start(f8cs[1][:], f8c[1])
            nc.sync.dma_start(combs[1][:, CB:], comb[1, :, CB:])
            load_x(1, 4, 8)
            for kg in range(2, KT // GRP):
                for h in range(HPC):
                    load_x(h, kg * GRP, (kg + 1) * GRP)

            # Absorb the comb-DMA waits on both drain engines so psum-drain
            # ops only ever wait on the PE semaphore (walrus allows a single
            # sync wait per compute instruction).  The Act copy also hoists
            # the one-time ACT_TABLE_LOAD into the start-up bubble.
            bscrV = sp.tile([128, 2 * 32], DT, tag="bscrV", name="bscrV")
            bscrA = sp.tile([128, 2 * 32], DT, tag="bscrA", name="bscrA")
            for h in range(HPC):
                nc.vector.tensor_copy(bscrV[:, 32 * h:32 * h + 32],
                                      combs[h][:, CA:CB])
                nc.scalar.copy(bscrA[:, 32 * h:32 * h + 32],
                               combs[h][:, CA:CB])

            def w(h, d):        # fp16 Toeplitz weight block for offset d
                c = d * 128 if d < GRP else CB + (d - GRP) * 128
                return combs[h][:, c:c + 128]

            def bias(h, tb):    # fp32 per-partition bias column for tile tb
                return combs[h][:, CA:CB].bitcast(F32)[:, tb:tb + 1]

            def fp8_pair(h, tb):
                # slots [Mq_{tb-3} | Mq_{tb-2}] against [x3q | x2q]:
                # psum[tb] += Mq_{tb-3}.T @ xq_3 + Mq_{tb-2}.T @ xq_2
                lhsT = f8cs[h][:, 128 * (tb - 3):128 * (tb - 1)].rearrange(
                    "p (two m) -> p two m", two=2)
                rhs = f8cs[h][:, S:S + 2 * R].rearrange(
                    "p (two n) -> p two n", two=2)
                return lhsT, rhs

            def use_fp8(tb, k):
                return FP8_MIX and tb >= 4 and k in FP8_KS

            for g in range(KT // GRP):
                tbs = range(GRP * g, GRP * (g + 1))
                for h in range(HPC):
                    pool = psA if h == 0 else psB
                    ps = {tb: pool.tile([128, R], F32, tag="acc",
                                        name=f"acc{h}_{tb}")
                          for tb in tbs}
                    if g == 0 or not FP8_MIX:
                        # k=0 starts: g0 gates only on the comb_a DMA; later
                        # groups leave the bank-free sem as the only wait.
                        for tb in tbs:
                            nc.tensor.matmul(ps[tb][:], w(h, tb), xs[h][0],
                                             start=True, stop=(tb == 0))
                    else:
                        # fp8 DoubleRow pair as each tile's start matmul.
                        for tb in tbs:
                            lhsT, rhs = fp8_pair(h, tb)
                            nc.tensor.matmul(
                                ps[tb][:], lhsT, rhs, start=True, stop=False,
                                perf_mode=mybir.MatmulPerfMode.DoubleRow)
                        for tb in tbs:
                            nc.tensor.matmul(ps[tb][:], w(h, tb), xs[h][0],
                                             start=False, stop=False)
                    last = (h == HPC - 1 and g == KT // GRP - 1)
                    if not last:
                        for d in range(tbs.stop - 1):
                            for tb in range(max(tbs.start, d + 1), tbs.stop):
                                if use_fp8(tb, tb - d):
                                    continue
                                nc.tensor.matmul(
                                    ps[tb][:], w(h, d), xs[h][tb - d],
                                    start=False, stop=(d == tb - 1))
                        # Drain psum (+bias) on DVE, staged output DMAs on
                        # Sync.
                        for p0, p1 in ((0, 2), (2, 4)):
                            w_ = p1 - p0
                            o = op.tile([128, w_ * R], DT, tag="o",
                                        name=f"o{h}_{g}_{p0}")
                            for j, tb in enumerate(tbs[p0:p1]):
                                nc.vector.tensor_scalar_add(
                                    o[:, j * R:(j + 1) * R], ps[tb][:],
                                    bias(h, tb))
                            nc.sync.dma_start(
                                out[h, g][:, p0 * R:p1 * R], o[:])
                    else:
                        # Final group: finish tiles high-to-low so the
                        # last-stopping tile (t12) drains as two half-tiles
                        # on DVE + Act in parallel, each chased by its own
                        # output DMA; earlier tiles stream out while t12 is
                        # still accumulating.
                        for tb in reversed(tbs):
                            for d in range(tb):
                                if use_fp8(tb, tb - d):
                                    continue
                                nc.tensor.matmul(
                                    ps[tb][:], w(h, d), xs[h][tb - d],
                                    start=False, stop=(d == tb - 1))
                        t0 = tbs.start
                        for j, tb in enumerate(reversed(tbs[1:])):
                            o = op.tile([128, R], DT, tag="o",
                                        name=f"o{h}_{g}_{tb}")
                            if j % 2 == 0:
                                nc.vector.tensor_scalar_add(
                                    o[:], ps[tb][:], bias(h, tb))
                                nc.sync.dma_start(
                                    out[h, g][:, (tb - t0) * R:
                                               (tb - t0 + 1) * R], o[:])
                            else:
                                nc.scalar.add(o[:], ps[tb][:], bias(h, tb))
                                nc.scalar.dma_start(
                                    out[h, g][:, (tb - t0) * R:
                                               (tb - t0 + 1) * R], o[:])
                        hR = R // 2
                        oA2 = sp.tile([128, hR], DT, tag="oA2", name="oA2")
                        oD2 = sp.tile([128, hR], DT, tag="oD2", name="oD2")
                        nc.scalar.add(oA2[:], ps[t0][:, 0:hR], bias(h, t0))
                        nc.scalar.dma_start(out[h, g][:, 0:hR], oA2[:])
                        nc.vector.tensor_scalar_add(oD2[:], ps[t0][:, hR:R],
                                                    bias(h, t0))
                        nc.sync.dma_start(out[h, g][:, hR:R], oD2[:])
    nc.compile()
    return nc


def _get_nc():
    global _CACHED_NC
    if _CACHED_NC is None:
        _CACHED_NC = _build_bass()
    return _CACHED_NC


def _toeplitz_rows(w_row):
    """(S,) weight -> (128, S) array T, T[i, c] = w[c-i] (0 where c < i)."""
    wpad = np.concatenate([np.zeros(127, np.float32),
                           np.asarray(w_row, np.float32)])
    sw = np.lib.stride_tricks.sliding_window_view(wpad, S)   # (128, S)
    return sw[127::-1]


def run(x, weight, bias, trace=False, trace_kwargs=None, trace_cores=None):
    x = np.ascontiguousarray(np.asarray(x, np.float32))
    weight = np.asarray(weight, np.float32)
    bias = np.asarray(bias, np.float32)

    xg = x.reshape(B * H, D, S).reshape(H, B, D, S)   # view, no copy

    in_maps = []
    for c in range(NCORES):
        xtt = np.empty((HPC, S, R), NPDT)
        combv = np.empty((HPC, 128, CW), NPDT)
        if FP8_MIX:
            f8cv = np.empty((HPC, 128, S + 2 * R), NP8)
        for i in range(HPC):
            m = HPC * c + i
            xT = np.ascontiguousarray(xg[m].reshape(R, S).T)  # (S, R) f32
            xtt[i] = xT
            tw = _toeplitz_rows(weight[m])                    # (128, S) f32
            combv[i, :, :GRP * 128] = tw[:, :GRP * 128]
            combv[i, :, GRP * 128:CA] = xT[:128]
            combv[i, :, CA:CB] = (
                np.ascontiguousarray(bias[m].reshape(KT, 128).T)
                .view(np.float16))
            combv[i, :, CB:] = tw[:, GRP * 128:]
            if FP8_MIX:
                f8cv[i, :, :S] = tw.astype(NP8)
                f8cv[i, :, S:S + R] = xT[3 * 128:4 * 128].astype(NP8)
                f8cv[i, :, S + R:] = xT[2 * 128:3 * 128].astype(NP8)
        im = {"xt": xtt, "comb": combv}
        if FP8_MIX:
            im["f8c"] = f8cv
        in_maps.append(im)

    nc = _get_nc()
    kw = {}
    if trace:
        kw["trace"] = True
        if trace_kwargs:
            kw["trace_kwargs"] = trace_kwargs
        if trace_cores is not None:
            kw["trace_cores"] = trace_cores
    res = run_bass_kernel_spmd(nc, in_maps, core_ids=list(range(NCORES)), **kw)

    outg = np.empty((H, B, D, S), np.float32)
    for c in range(NCORES):
        o = res.results[c]["out"].astype(np.float32)
        o = o.reshape(HPC, KT // GRP, 128, GRP, R)
        for i in range(HPC):
            m = HPC * c + i
            ct = o[i].transpose(0, 2, 1, 3).reshape(S, R)   # (t, r)
            outg[m] = ct.T.reshape(B, D, S)
    return outg.reshape(B, E, S), res


def kernel(x, weight, bias):
    out, _ = run(x, weight, bias, trace=False)
    return out


# revision 32
# speedup vs baseline: 1.1563x; 1.1563x over previous
"""MultiHeadMixer Trainium2 kernel (v4 = proven v1 skeleton + partial fp8).

Math (matches the reference):
  x: (B=8, E=1024, S=2048), weight: (H=16, S), bias: (H, S)
  out[m, r, t] = sum_{s<=t} xg[m, r, s] * weight[m, t-s] + bias[m, t]

Per head C = A @ M with M upper-triangular Toeplitz; every 128x128 block of
M depends only on d = t_tile - s_tile and is a column slice of the (128, S)
shifted-weight array T, T[i, c] = w[c - i].  Device computation per head:
  psum[tb] += T(d).T @ xT[k],  d = tb - k >= 0
with xT the transposed x staged per 128-row s-tile (moving free dim N=512).
136 upper-triangular blocks per head; lower-triangle blocks skipped.

v4 vs v1 (76.9us):
  * Blocks k in {2,3} for tb >= 4 run as one fp8e4 DoubleRow matmul per
    output tile (2 blocks per 512-cycle matmul = 2x MAC rate).  24/136 of
    the MACs in fp8 -> rel err ~1.6e-2 (gate 2e-2), saves 12 matmuls/head
    (~5.2us of PE time).  The fp8 matmul doubles as each tile's start in
    g1..g3 so it carries only the psum-bank wait.
  * Sync DMA order tuned (x h0 k1-3 right after comb_a h0, comb_a h1
    earlier) to close v1's early supply gaps.  All DMAs stay on the Sync
    queue: spreading them over Act/GpSimd queues starves the DMA-semaphore
    pool and serializes on recycle waits (v2/v3 regressions).
  * Tail: the last output tile drains as two 256-col halves, one on the
    Act engine (which issues its own output DMA back-to-back), one on DVE.

Sharding: head-parallel, 2 heads per core across 8 cores, no collectives.
"""

import numpy as np
import ml_dtypes

import concourse.bass as bass
import concourse.mybir as mybir
import concourse.tile as tile
from concourse import bacc
from concourse.bass_utils import run_bass_kernel_spmd

B, E, S, H = 8, 1024, 2048, 16
D = E // H            # 64   per-head hidden dim
NCORES = 8
HPC = H // NCORES     # 2    heads per core
R = B * D             # 512  rows per head (moving free dim)
KT = S // 128         # 16   128-wide tiles along the sequence axis
GRP = 4               # output tiles per PSUM group (4 banks per head)

DT = mybir.dt.float16
F8 = mybir.dt.float8e4
F32 = mybir.dt.float32
NPDT = np.float16
NP8 = ml_dtypes.float8_e4m3

import os as _os
FP8_MIX = _os.environ.get("FP8_MIX", "1") == "1"
FP8_KS = (2, 3)           # fp8 covers k in FP8_KS for tb >= 4
FP8_KS2 = (12, 13)        # plus k in FP8_KS2 for tb >= 13 (g3 only)

CA = GRP * 128 + R        # comb_a width: [Td0..3 | x0]

_CACHED_NC = None


def _ensure_axon_hooks_stub():
    try:
        import antenv.axon_hooks  # noqa: F401
    except ImportError:
        import sys
        import types
        import antenv
        mod = types.ModuleType("antenv.axon_hooks")
        mod._hook = None
        mod.set_axon_ntff_profile_hook = lambda h: setattr(mod, "_hook", h)
        mod.get_axon_ntff_profile_hook = lambda: mod._hook
        sys.modules["antenv.axon_hooks"] = mod
        antenv.axon_hooks = mod


_ensure_axon_hooks_stub()


def _build_bass():
    nc = bacc.Bacc()
    xt = nc.dram_tensor("xt", [HPC, S, R], DT, kind="ExternalInput")
    comb = nc.dram_tensor("comb", [HPC, 128, S + R], DT, kind="ExternalInput")
    biast = nc.dram_tensor("biast", [HPC, 128, KT], F32, kind="ExternalInput")
    if FP8_MIX:
        f8c = nc.dram_tensor("f8c", [HPC, 128, S + 4 * R], F8,
                             kind="ExternalInput")
    out = nc.dram_tensor("out", [HPC, KT // GRP, 128, GRP * R], DT,
                         kind="ExternalOutput")

    with tile.TileContext(nc) as tc:
        with (
            tc.tile_pool(name="xp", bufs=HPC * 4 + 1) as xp,
            tc.tile_pool(name="tp", bufs=HPC) as tp,
            tc.tile_pool(name="fp", bufs=HPC) as fpp,
            tc.tile_pool(name="bp", bufs=HPC) as bp,
            tc.tile_pool(name="op", bufs=HPC * (KT // GRP)) as op,
            tc.tile_pool(name="sp", bufs=1) as sp,
            tc.tile_pool(name="psA", bufs=GRP, space="PSUM") as psA,
            tc.tile_pool(name="psB", bufs=GRP, space="PSUM") as psB,
        ):
            # PE warm-up: HAM clock gate needs ~3.4us of sustained PE
            # activity; the PE idles ~10us waiting for the first DMA anyway.
            warm = sp.tile([128, R + 128], DT, tag="warm", name="warm")
            nc.gpsimd.memset(warm[:], 0)
            wps = psA.tile([128, R], F32, tag="acc", name="warm_ps")
            NWARM = 7
            for i in range(NWARM):
                nc.tensor.matmul(wps[:], warm[:, R:R + 128], warm[:, 0:R],
                                 start=(i == 0), stop=(i == NWARM - 1))

            combs, f8cs, xs = [], [], []
            bias_sb = bp.tile([128, HPC * KT], F32, tag="bias", name="bias")
            biases = [bias_sb[:, h * KT:(h + 1) * KT] for h in range(HPC)]
            for h in range(HPC):
                combs.append(tp.tile([128, S + R], DT, tag="T",
                                     name=f"comb{h}"))
                if FP8_MIX:
                    f8cs.append(fpp.tile([128, S + 4 * R], F8, tag="T8",
                                         name=f"f8c{h}"))
                xs.append({0: combs[h][:, GRP * 128:CA]})

            def load_x(h, k0, k1):
                wd = (k1 - k0) * R
                xb = xp.tile([128, wd], DT, tag="x4", name=f"xb{h}_{k0}")
                src = xt[h, k0 * 128:k1 * 128, :].rearrange(
                    "(k p) r -> p k r", p=128)
                dst = xb[:].rearrange("p (k r) -> p k r", k=k1 - k0)
                nc.sync.dma_start(dst, src)
                for k in range(k0, k1):
                    xs[h][k] = xb[:, (k - k0) * R:(k - k0 + 1) * R]

            # All input DMAs on the Sync queue, ordered by consumption time.
            # x h0 k1 is split out so the g0 k-ascending loop only gates on
            # a 128KB transfer; both heads' bias columns ride one DMA.
            nc.sync.dma_start(combs[0][:, :CA], comb[0, :, :CA])
            load_x(0, 1, 2)
            nc.sync.dma_start(combs[1][:, :CA], comb[1, :, :CA])
            load_x(0, 2, 4)
            load_x(1, 1, 4)
            nc.sync.dma_start(
                bias_sb[:].rearrange("p (h k) -> p h k", h=HPC),
                biast[:, :, :].rearrange("h p k -> p h k"))
            if FP8_MIX:
                nc.sync.dma_start(f8cs[0][:], f8c[0])
            nc.sync.dma_start(combs[0][:, CA:], comb[0, :, CA:])
            load_x(0, 4, 8)
            if FP8_MIX:
                nc.sync.dma_start(f8cs[1][:], f8c[1])
            nc.sync.dma_start(combs[1][:, CA:], comb[1, :, CA:])
            load_x(1, 4, 8)
            for kg in range(2, KT // GRP):
                for h in range(HPC):
                    load_x(h, kg * GRP, (kg + 1) * GRP)

            # Absorb the bias-DMA waits on both drain engines so psum-drain
            # ops only ever wait on the PE semaphore (walrus allows a single
            # sync wait per compute instruction).  The Act copy also hoists
            # the one-time ACT_TABLE_LOAD into the start-up bubble.
            bscrV = sp.tile([128, 2 * KT], F32, tag="bscrV", name="bscrV")
            bscrA = sp.tile([128, KT], F32, tag="bscrA", name="bscrA")
            for h in range(HPC):
                nc.vector.tensor_copy(bscrV[:, KT * h:KT * h + KT],
                                      biases[h][:])
            nc.scalar.copy(bscrA[:], biases[1][:])

            def w(h, d):        # fp16 Toeplitz weight block for offset d
                c = d * 128 if d < GRP else R + d * 128
                return combs[h][:, c:c + 128]

            def fp8_pair(h, tb):
                # slots [Mq_{tb-3} | Mq_{tb-2}] against [x3q | x2q]:
                # psum[tb] += Mq_{tb-3}.T @ xq_3 + Mq_{tb-2}.T @ xq_2
                lhsT = f8cs[h][:, 128 * (tb - 3):128 * (tb - 1)].rearrange(
                    "p (two m) -> p two m", two=2)
                rhs = f8cs[h][:, S:S + 2 * R].rearrange(
                    "p (two n) -> p two n", two=2)
                return lhsT, rhs

            def fp8_pair2(h, tb):
                # slots [Mq_{tb-13} | Mq_{tb-12}] against [x13q | x12q]
                lhsT = f8cs[h][:, 128 * (tb - 13):128 * (tb - 11)].rearrange(
                    "p (two m) -> p two m", two=2)
                rhs = f8cs[h][:, S + 2 * R:S + 4 * R].rearrange(
                    "p (two n) -> p two n", two=2)
                return lhsT, rhs

            def use_fp8(tb, k):
                return FP8_MIX and ((tb >= 4 and k in FP8_KS)
                                    or (tb >= 13 and k in FP8_KS2))

            for g in range(KT // GRP):
                tbs = range(GRP * g, GRP * (g + 1))
                for h in range(HPC):
                    pool = psA if h == 0 else psB
                    ps = {tb: pool.tile([128, R], F32, tag="acc",
                                        name=f"acc{h}_{tb}")
                          for tb in tbs}
                    if g == 0 or not FP8_MIX:
                        # k=0 starts: g0 gates only on the comb_a DMA; later
                        # groups leave the bank-free sem as the only wait.
                        for tb in tbs:
                            nc.tensor.matmul(ps[tb][:], w(h, tb), xs[h][0],
                                             start=True, stop=(tb == 0))
                    if g == 0:
                        # k-ascending so the first x DMA only gates k=1.
                        for k in range(1, GRP):
                            for tb in range(k, GRP):
                                nc.tensor.matmul(
                                    ps[tb][:], w(h, tb - k), xs[h][k],
                                    start=False, stop=(k == tb))
                    else:
                        if FP8_MIX:
                            # fp8 DoubleRow pair as each tile's start.
                            for tb in tbs:
                                lhsT, rhs = fp8_pair(h, tb)
                                nc.tensor.matmul(
                                    ps[tb][:], lhsT, rhs, start=True,
                                    stop=False,
                                    perf_mode=mybir.MatmulPerfMode.DoubleRow)
                            for tb in tbs:
                                nc.tensor.matmul(ps[tb][:], w(h, tb),
                                                 xs[h][0], start=False,
                                                 stop=False)
                            for tb in tbs:
                                if tb >= 13:
                                    lhsT, rhs = fp8_pair2(h, tb)
                                    nc.tensor.matmul(
                                        ps[tb][:], lhsT, rhs, start=False,
                                        stop=False,
                                        perf_mode=
                                        mybir.MatmulPerfMode.DoubleRow)
                        for d in range(tbs.stop - 1):
                            for tb in range(max(tbs.start, d + 1), tbs.stop):
                                if use_fp8(tb, tb - d):
                                    continue
                                nc.tensor.matmul(
                                    ps[tb][:], w(h, d), xs[h][tb - d],
                                    start=False, stop=(d == tb - 1))

                    # Drain psum (+bias) on DVE, big staged output DMAs on
                    # Sync; the very last tile splits across Act + DVE so
                    # the tail chain is one small drain + a same-engine DMA.
                    last = (h == HPC - 1 and g == KT // GRP - 1)
                    t0, t1, t2, t3 = tbs
                    if not last:
                        pieces = ((0, 2), (2, 4))
                    else:
                        pieces = ((0, 2), (2, 3))
                    for p0, p1 in pieces:
                        w_ = p1 - p0
                        o = op.tile([128, w_ * R], DT, tag="o",
                                    name=f"o{h}_{g}_{p0}")
                        for j, tb in enumerate(tbs[p0:p1]):
                            nc.vector.tensor_scalar_add(
                                o[:, j * R:(j + 1) * R], ps[tb][:],
                                biases[h][:, tb:tb + 1])
                        nc.sync.dma_start(
                            out[h, g][:, p0 * R:p1 * R], o[:])
                    if last:
                        hR = R // 2
                        oA2 = sp.tile([128, hR], DT, tag="oA2", name="oA2")
                        oD2 = sp.tile([128, hR], DT, tag="oD2", name="oD2")
                        nc.scalar.add(oA2[:], ps[t3][:, 0:hR],
                                      biases[h][:, t3:t3 + 1])
                        nc.scalar.dma_start(
                            out[h, g][:, 3 * R:3 * R + hR], oA2[:])
                        nc.vector.tensor_scalar_add(oD2[:], ps[t3][:, hR:R],
                                                    biases[h][:, t3:t3 + 1])
                        nc.sync.dma_start(
                            out[h, g][:, 3 * R + hR:4 * R], oD2[:])
    nc.compile()
    return nc


def _get_nc():
    global _CACHED_NC
    if _CACHED_NC is None:
        _CACHED_NC = _build_bass()
    return _CACHED_NC


def _toeplitz_rows(w_row):
    """(S,) weight -> (128, S) array T, T[i, c] = w[c-i] (0 where c < i)."""
    wpad = np.concatenate([np.zeros(127, np.float32),
                           np.asarray(w_row, np.float32)])
    sw = np.lib.stride_tricks.sliding_window_view(wpad, S)   # (128, S)
    return sw[127::-1]


def run(x, weight, bias, trace=False, trace_kwargs=None, trace_cores=None):
    x = np.ascontiguousarray(np.asarray(x, np.float32))
    weight = np.asarray(weight, np.float32)
    bias = np.asarray(bias, np.float32)

    xg = x.reshape(B * H, D, S).reshape(H, B, D, S)   # view, no copy

    in_maps = []
    for c in range(NCORES):
        xtt = np.empty((HPC, S, R), NPDT)
        combv = np.empty((HPC, 128, S + R), NPDT)
        biastv = np.empty((HPC, 128, KT), np.float32)
        if FP8_MIX:
            f8cv = np.empty((HPC, 128, S + 4 * R), NP8)
        for i in range(HPC):
            m = HPC * c + i
            xT = np.ascontiguousarray(xg[m].reshape(R, S).T)  # (S, R) f32
            xtt[i] = xT
            tw = _toeplitz_rows(weight[m])                    # (128, S) f32
            combv[i, :, :GRP * 128] = tw[:, :GRP * 128]
            combv[i, :, GRP * 128:CA] = xT[:128]
            combv[i, :, CA:] = tw[:, GRP * 128:]
            biastv[i] = np.ascontiguousarray(bias[m].reshape(KT, 128).T)
            if FP8_MIX:
                f8cv[i, :, :S] = tw.astype(NP8)
                f8cv[i, :, S:S + R] = xT[3 * 128:4 * 128].astype(NP8)
                f8cv[i, :, S + R:S + 2 * R] = xT[2 * 128:3 * 128].astype(NP8)
                f8cv[i, :, S + 2 * R:S + 3 * R] = (
                    xT[13 * 128:14 * 128].astype(NP8))
                f8cv[i, :, S + 3 * R:] = xT[12 * 128:13 * 128].astype(NP8)
        im = {"xt": xtt, "comb": combv, "biast": biastv}
        if FP8_MIX:
            im["f8c"] = f8cv
        in_maps.append(im)

    nc = _get_nc()
    kw = {}
    if trace:
        kw["trace"] = True
        if trace_kwargs:
            kw["trace_kwargs"] = trace_kwargs
        if trace_cores is not None:
            kw["trace_cores"] = trace_cores
    res = run_bass_kernel_spmd(nc, in_maps, core_ids=list(range(NCORES)), **kw)

    outg = np.empty((H, B, D, S), np.float32)
    for c in range(NCORES):
        o = res.results[c]["out"].astype(np.float32)
        o = o.reshape(HPC, KT // GRP, 128, GRP, R)
        for i in range(HPC):
            m = HPC * c + i
            ct = o[i].transpose(0, 2, 1, 3).reshape(S, R)   # (t, r)
            outg[m] = ct.T.reshape(B, D, S)
    return outg.reshape(B, E, S), res


def kernel(x, weight, bias):
    out, _ = run(x, weight, bias, trace=False)
    return out
